# revision 13
# baseline (speedup 1.0000x reference)
"""ChebConv (K=3) GNN message-passing kernel for Trainium2, 8 NeuronCores.

Strategy (graph/data parallel, dst-partitioned):
  - Nodes split into 8 contiguous ranges (12500/core); each core owns the
    output rows and in-edges of its range. Node storage is window-packed:
    node r of a core sits at SBUF [p=r%128, w=r//128]; the shared h table in
    HBM uses packed row index c*12544 + p*W + w so a full-slice DMA is one
    contiguous copy.
  - Edges are grouped per core by (src 32768-row chunk, 128-node dst window)
    with uniform cross-core tile budgets so the SPMD program is identical on
    all cores. Tiles are chunk-major, enabling big gather calls.
  - Message gather: 8192-index SWDGE dma_gather calls (64 tiles per call,
    int16 indices) pull h[src] rows (bf16, 256B) from the HBM table into
    [128 edges x F] SBUF tiles. Big calls amortize the ~1us/call Q7
    descriptor-generation overhead that dominated the previous version.
  - Aggregation: per 128-edge tile, a host-built one-hot S tile (fp8)
    maps edges to window rows via the TensorEngine; per-window PSUM partials
    are flushed into an fp16 per-core accumulator across chunks.
  - Chebyshev recurrence in bf16 on-chip; h tables rebuilt per hop and
    exchanged with AllGather.
Host-side work is limited to graph restructuring (edge partition / sort /
padding, degree counting, one-hot layout) - all floating-point math on node
features happens on device.
"""

import os
import sys

import numpy as np

sys.path.insert(0, "/opt/trn_rl_repo")

from contextlib import ExitStack


def _ensure_ntff_hook():
    """The agent image's antenv lacks axon_hooks; synthesize the module so
    run_bass_kernel_spmd(trace=True) can reach the NTFF profiler."""
    import types

    if "antenv.axon_hooks" in sys.modules:
        return
    try:
        import trn_agent_boot.trn_boot as tb

        hook = tb._ntff_profile_via_ctypes("/opt/axon/libaxon_pjrt.so")
    except Exception:
        hook = None
    mod = types.ModuleType("antenv.axon_hooks")
    state = {"hook": hook}
    mod.get_axon_ntff_profile_hook = lambda: state["hook"]
    mod.set_axon_ntff_profile_hook = lambda h: state.update(hook=h)
    sys.modules["antenv.axon_hooks"] = mod


_ensure_ntff_hook()

import concourse.bacc as bacc
import concourse.bass as bass
import concourse.tile as tile
from concourse import mybir
from concourse.bass_utils import run_bass_kernel_spmd

P = 128
F = 128
K = 3


class Cfg:
    def __init__(self, N, E, n_cores, nq=4, gmax=None,
                 s_dtype=mybir.dt.float8e4):
        self.N = N
        self.E = E
        self.NC = n_cores
        assert N % n_cores == 0
        self.NPC = N // n_cores
        self.W = (self.NPC + P - 1) // P      # windows per core
        self.WP = self.W * P                  # packed rows per core
        self.NPACK = self.NC * self.WP        # packed rows total
        # Window-quarter chunking: the h table is packed quarter-major so
        # that AllGather piece q delivers exactly chunk q's rows. Chunk q =
        # all cores' quarter-q windows; each spmm's gathers for chunk q
        # depend only on AG piece q, letting desc-gen flow across the
        # h-table barriers.
        self.NK = nq
        base, rem = divmod(self.W, nq)
        self.Wq = [base + (1 if i < rem else 0) for i in range(nq)]
        self.qoff = np.concatenate([[0], np.cumsum(self.Wq)]).astype(int)
        self.chunk_rows = [self.NC * wq * P for wq in self.Wq]
        self.chunkbase = np.concatenate(
            [[0], np.cumsum(self.chunk_rows)]
        ).astype(int)
        assert max(self.chunk_rows) <= 32768  # int16 gather idx range
        self.GMAX = gmax if gmax is not None else int(
            os.environ.get("CHEB_GMAX", "8")
        )
        self.s_dtype = s_dtype

    def quarter_of_window(self, w):
        return int(np.searchsorted(self.qoff, w, side="right") - 1)


FULL = Cfg(100000, 3200000, 8)
NUM_QUEUES = int(os.environ.get("CHEB_QUEUES", "4"))


def packed_index(cfg, u):
    """Global node id -> packed h-table row index (quarter-major)."""
    c = u // cfg.NPC
    r = u - c * cfg.NPC
    p = r % P
    w = r // P
    q = np.searchsorted(cfg.qoff, w, side="right") - 1
    wq = np.asarray(cfg.Wq)[q]
    return cfg.chunkbase[q] + c * wq * P + p * wq + (w - cfg.qoff[q])


def preprocess(cfg, src, dst):
    """Partition edges by (dst core, src chunk, dst window); assign 128-edge
    tile slots with static per-(chunk, window) budgets (max over cores).

    Returns (B, deg, per_core):
      B:   [NK, W] int tile budgets (chunk-major tile layout, w-minor)
      deg: [N] float32 in-degrees
      per_core[c]: idx16 [128, TOT*8] int16 stream, s [128, TOT*F] one-hot
    """
    N, NC, NPC, W, NK = cfg.N, cfg.NC, cfg.NPC, cfg.W, cfg.NK
    src = np.asarray(src).astype(np.int64)
    dst = np.asarray(dst).astype(np.int64)
    deg = np.bincount(dst, minlength=N).astype(np.float32)

    gs_all = packed_index(cfg, src)

    cores = []
    cnts = np.zeros((NC, NK, W), dtype=np.int64)
    for c in range(NC):
        base = c * NPC
        sel = (dst >= base) & (dst < base + NPC)
        eg = gs_all[sel]
        rd = dst[sel] - base
        wof = rd >> 7
        dl = rd & 127
        ch = np.searchsorted(cfg.chunkbase, eg, side="right") - 1
        loc = eg - cfg.chunkbase[ch]
        order = np.lexsort((wof, ch))
        eg, wof, dl, ch, loc = (
            eg[order], wof[order], dl[order], ch[order], loc[order]
        )
        np.add.at(cnts[c], (ch, wof), 1)
        cores.append((loc, wof, dl, ch))

    B = np.ceil(cnts.max(axis=0) / P).astype(np.int64)  # [NK, W]
    flat = B.reshape(-1)
    toff = np.concatenate([[0], np.cumsum(flat)[:-1]]).reshape(NK, W)
    TOT = int(flat.sum())

    s_np_dtype = mybir.dt.np(cfg.s_dtype)
    one = np.ones(1, dtype=s_np_dtype)[0]
    per_core = []
    for c in range(NC):
        loc, wof, dl, ch = cores[c]
        seg_sizes = cnts[c].reshape(-1)
        seg_starts = np.concatenate([[0], np.cumsum(seg_sizes)[:-1]])
        seg_id = ch * W + wof
        rank = np.arange(len(loc)) - seg_starts[seg_id]
        gslot = toff[ch, wof] * P + rank

        idx16 = np.zeros((16, TOT * 8), dtype=np.int16)
        idx16[gslot % 16, gslot // 16] = loc.astype(np.int16)
        idx16 = np.tile(idx16, (8, 1))

        s_stream = np.zeros((P, TOT * F), dtype=s_np_dtype)
        s_stream[gslot % P, (gslot // P) * F + dl] = one
        per_core.append({"idx16": idx16, "s": s_stream})
    return B, deg, per_core


def pack_pw(arr_slice, W, fill=0.0):
    """[NPC, ...] node-major -> [128, W * inner] partition/window packed."""
    NPC = arr_slice.shape[0]
    inner = arr_slice.shape[1] if arr_slice.ndim > 1 else 1
    pad_rows = W * P - NPC
    a = arr_slice.reshape(NPC, inner)
    if pad_rows:
        a = np.concatenate(
            [a, np.full((pad_rows, inner), fill, dtype=a.dtype)], axis=0
        )
    return np.ascontiguousarray(
        a.reshape(W, P, inner).transpose(1, 0, 2).reshape(P, W * inner)
    )


def unpack_pw(a, W, NPC, inner):
    """[128, W * inner] -> [NPC, inner]."""
    return np.ascontiguousarray(
        a.reshape(P, W, inner).transpose(1, 0, 2).reshape(W * P, inner)[:NPC]
    )


def plan_calls(cfg, B):
    """Static gather-call plan. Returns per chunk k:
    (ct0, ct1, calls, groups) with calls = fixed-size [(t0, nt)] runs and
    groups = [(w, ts, u)]; a group may span up to 3 consecutive calls."""
    NK, W = B.shape
    running = 0
    plans = []
    for k in range(NK):
        ct0 = running
        groups = []
        for w in range(W):
            if B[k, w] > 0:
                groups.append((w, int(running), int(B[k, w])))
                running += int(B[k, w])
        ct1 = running
        calls = [
            (t0, min(cfg.GMAX, ct1 - t0))
            for t0 in range(ct0, ct1, cfg.GMAX)
        ]
        plans.append((ct0, ct1, calls, groups))
    return plans


def build_program(cfg, B):
    NC, W, NK = cfg.NC, cfg.W, cfg.NK
    B = np.asarray(B)
    plans = plan_calls(cfg, B)
    TOT = int(B.sum())
    CT_MAX = max(ct1 - ct0 for ct0, ct1, _, _ in plans)
    GMAX = cfg.GMAX
    dt = mybir.dt
    nc = bacc.Bacc(
        "TRN2",
        target_bir_lowering=False,
        debug=False,
        enable_asserts=False,
        num_devices=NC,
        num_swdge_queues=NUM_QUEUES,
        # 32KB/partition desc-ring carveout -> 256 descs per SWDGE queue ring
        # (default 16KB -> 128). A 1024-idx gather call writes 65 descs per
        # DMA ring; 128-deep rings fit only one call per queue, so the Q7
        # desc-gen stalls in await_space whenever the DMA engines lag a call.
        dynamic_dma_scratch_size=int(os.environ.get("CHEB_DSCRATCH", "32768")),
    )

    feat_pw = nc.dram_tensor("feat_pw", [P, W * F], dt.float32, kind="ExternalInput")
    deg_pw = nc.dram_tensor("deg_pw", [P, W], dt.float32, kind="ExternalInput")
    lam_d = nc.dram_tensor("lam_d", [1, 1], dt.float32, kind="ExternalInput")
    wmat_d = nc.dram_tensor("wmat_d", [K * F, F], dt.float32, kind="ExternalInput")
    bvec_d = nc.dram_tensor("bvec_d", [1, F], dt.float32, kind="ExternalInput")
    ident_d = nc.dram_tensor("ident_d", [P, P], dt.float32, kind="ExternalInput")
    idx_d = nc.dram_tensor("idx_d", [P, TOT * 8], dt.int16, kind="ExternalInput")
    s_d = nc.dram_tensor("s_d", [P, TOT * F], cfg.s_dtype, kind="ExternalInput")
    out_pw = nc.dram_tensor("out_pw", [P, W * F], dt.float32, kind="ExternalOutput")

    h_slice = [
        [
            nc.dram_tensor(
                f"h{i}_slice_{q}", [cfg.Wq[q] * P, F], dt.bfloat16,
                kind="Internal",
            )
            for q in range(NK)
        ]
        for i in range(2)
    ]
    h_full = [
        [
            nc.dram_tensor(
                f"h{i}_full_{q}", [cfg.chunk_rows[q], F], dt.bfloat16,
                kind="Internal", addr_space="Shared",
            )
            for q in range(NK)
        ]
        for i in range(2)
    ]
    groups_cc = [list(range(NC))]

    with tile.TileContext(nc) as tc, ExitStack() as ctx:
        const = ctx.enter_context(tc.tile_pool(name="const", bufs=1))
        ipool = ctx.enter_context(tc.tile_pool(name="ipool", bufs=2))
        # Tile rotates Pool-DMA completion sems over 8 DMASW lanes (emission
        # order). bufs=6 keeps at most 6 gathers in flight -> no two live
        # gathers share a lane. bufs=8 DID race (two in-flight gathers alias
        # a lane; matmuls read against the wrong completion: rel err 0.017).
        gbufs = int(os.environ.get("CHEB_GBUFS", "6"))
        spool = ctx.enter_context(tc.tile_pool(name="spool", bufs=gbufs))
        mpool = ctx.enter_context(tc.tile_pool(name="mpool", bufs=gbufs))
        vpool = ctx.enter_context(tc.tile_pool(name="vpool", bufs=3))
        opool = ctx.enter_context(tc.tile_pool(name="opool", bufs=3))
        fpool = ctx.enter_context(tc.tile_pool(name="fpool", bufs=2))
        apool = ctx.enter_context(tc.tile_pool(name="apool", bufs=2, space="PSUM"))
        ppool = ctx.enter_context(tc.tile_pool(name="ppool", bufs=2, space="PSUM"))
        rpool = ctx.enter_context(tc.tile_pool(name="rpool", bufs=2, space="PSUM"))
        mppool = ctx.enter_context(tc.tile_pool(name="mppool", bufs=1, space="PSUM"))

        # ---- constants / persistent state ----
        # NOTE: keep nc.gpsimd DMA usage to dma_gather ONLY — Tile rotates
        # SWDGE sem lanes per Pool-DMA in emission order and each lane is
        # locked to one queue; any extra Pool DMA breaks the queue rotation.
        tx0 = const.tile([P, W, F], dt.bfloat16)
        tx1 = const.tile([P, W, F], dt.bfloat16)
        scratch = const.tile([P, W, F], dt.float16)  # agg (fp16) / hb (bf16)
        agg = scratch[:]
        hb = scratch[:].bitcast(dt.bfloat16)
        tx0_flat = tx0[:].rearrange("p w f -> p (w f)")
        wsb = []
        for k in range(K):
            wtmp = vpool.tile([P, F], dt.float32, tag="wtmp")
            nc.sync.dma_start(
                out=wtmp[:], in_=wmat_d.ap()[k * F : (k + 1) * F, :]
            )
            wk = const.tile([P, F], dt.bfloat16, tag=f"wsb{k}")
            nc.vector.tensor_copy(out=wk[:], in_=wtmp[:])
            wsb.append(wk)
        ident = const.tile([P, P], dt.float32)
        nc.sync.dma_start(out=ident[:], in_=ident_d.ap())
        ident_b = const.tile([P, P], dt.bfloat16)
        nc.vector.tensor_copy(out=ident_b[:], in_=ident[:])
        ones_row = const.tile([1, P], dt.float32)
        nc.vector.memset(ones_row[:], 1.0)
        bvec_sb = const.tile([1, F], dt.float32)
        nc.sync.dma_start(out=bvec_sb[:], in_=bvec_d.ap())

        deg_sb = const.tile([P, W], dt.float32)
        nc.sync.dma_start(out=deg_sb[:], in_=deg_pw.ap())
        norm = const.tile([P, W], dt.float32)
        b128 = const.tile([P, F], dt.float32)
        nl = const.tile([P, W], dt.float32)  # norm * 2/lambda

        rec_deg = const.tile([P, W], dt.float32)
        nc.vector.reciprocal(rec_deg[:], deg_sb[:])
        nc.scalar.activation(
            norm[:], rec_deg[:], mybir.ActivationFunctionType.Sqrt
        )

        lam_sb = const.tile([1, 1], dt.float32)
        nc.sync.dma_start(out=lam_sb[:], in_=lam_d.ap())
        lam_half = const.tile([1, 1], dt.float32)
        nc.vector.tensor_scalar(
            lam_half[:], lam_sb[:], 0.5, None, mybir.AluOpType.mult
        )
        lap_sc = const.tile([1, 1], dt.float32)  # 2 / lambda_max
        nc.vector.reciprocal(lap_sc[:], lam_half[:])

        # broadcast 2/lambda to all partitions: ones[1,P]^T @ lap[1,1]
        lap_ps = mppool.tile([P, 1], dt.float32, space="PSUM", tag="mpsum")
        nc.tensor.matmul(
            lap_ps[:], lhsT=ones_row[:], rhs=lap_sc[:], start=True, stop=True
        )
        lap_bc = const.tile([P, 1], dt.float32)
        nc.vector.tensor_copy(out=lap_bc[:], in_=lap_ps[:])
        # bias broadcast to all partitions
        b_ps = mppool.tile([P, F], dt.float32, space="PSUM", tag="mpsum")
        nc.tensor.matmul(
            b_ps[:], lhsT=ones_row[:], rhs=bvec_sb[:], start=True, stop=True
        )
        nc.vector.tensor_copy(out=b128[:], in_=b_ps[:])

        nc.vector.tensor_scalar(
            nl[:], norm[:], lap_bc[:], None, mybir.AluOpType.mult
        )

        norm_b = norm[:].unsqueeze(-1).broadcast_to([P, W, F])

        def build_h_quarter(tx_tile, i, q):
            """h quarter = tx[:, q0:q1] * norm -> h_slice[i][q] -> AG piece."""
            q0, q1 = int(cfg.qoff[q]), int(cfg.qoff[q + 1])
            nc.vector.tensor_tensor(
                out=hb[:, q0:q1, :],
                in0=tx_tile[:, q0:q1, :],
                in1=norm_b[:, q0:q1, :],
                op=mybir.AluOpType.mult,
            )
            nc.sync.dma_start(
                out=h_slice[i][q].ap(),
                in_=hb[:, q0:q1, :].rearrange("p w f -> p (w f)"),
            )
            if os.environ.get("CHEB_NOCC", "0") == "1":
                for blk in range(0, cfg.Wq[q] * P, P):
                    tmp = vpool.tile([P, F], dt.bfloat16, tag="agtmp")
                    nc.sync.dma_start(
                        out=tmp[:], in_=h_slice[i][q].ap()[blk : blk + P, :]
                    )
                    nc.sync.dma_start(
                        out=h_full[i][q].ap()[blk : blk + P, :], in_=tmp[:]
                    )
                return
            nc.gpsimd.collective_compute(
                "AllGather",
                mybir.AluOpType.bypass,
                replica_groups=groups_cc,
                ins=[h_slice[i][q].ap()],
                outs=[h_full[i][q].ap()],
            )

        call_counter = [0]

        # first/last chunk containing tiles for each window (-1 = no edges)
        first_k = [-1] * W
        last_k = [-1] * W
        for w in range(W):
            ks = [k for k in range(NK) if B[k, w] > 0]
            if ks:
                first_k[w], last_k[w] = ks[0], ks[-1]

        def spmm(h_q_list, consume):
            """agg (fp16 scratch) = sum over edges; consume(w) after the
            window's last-chunk flush. Chunk k gathers from h_q_list[k]
            (written by AG piece k)."""
            for k in range(NK):
                ct0, ct1, calls, groups = plans[k]
                nt_chunk = ct1 - ct0
                if nt_chunk == 0:
                    continue
                i_t = ipool.tile([P, CT_MAX * 8], dt.int16, tag="i16")
                nc.sync.dma_start(
                    out=i_t[:, : nt_chunk * 8],
                    in_=idx_d.ap()[:, ct0 * 8 : ct1 * 8],
                )
                tiles_map = {}  # global tile idx -> (s_tile, m_tile, off)
                gi = 0  # next group to emit

                def emit_group(w, ts, u):
                    ps = apool.tile([P, F], dt.float32, space="PSUM", tag="agg")
                    for t in range(u):
                        s_t, m_t, off = tiles_map[ts + t]
                        nc.tensor.matmul(
                            ps[:],
                            lhsT=s_t[:, off, :],
                            rhs=m_t[:, off, :],
                            start=(t == 0),
                            stop=(t == u - 1),
                        )
                    if k == first_k[w]:
                        nc.vector.tensor_copy(out=agg[:, w, :], in_=ps[:])
                    else:
                        nc.vector.tensor_tensor(
                            out=agg[:, w, :], in0=agg[:, w, :], in1=ps[:],
                            op=mybir.AluOpType.add,
                        )
                    if k == last_k[w]:
                        consume(w)

                for (t0, nt) in calls:
                    s_t = spool.tile([P, GMAX, F], cfg.s_dtype, tag="s")
                    nc.sync.dma_start(
                        out=s_t[:, :nt, :].rearrange("p t f -> p (t f)"),
                        in_=s_d.ap()[:, t0 * F : (t0 + nt) * F],
                    )
                    m_t = mpool.tile([P, GMAX, F], dt.bfloat16, tag="msg")
                    nc.gpsimd.dma_gather(
                        out_ap=m_t[:, :nt, :],
                        in_ap=h_q_list[k].ap(),
                        idxs_ap=i_t[:, (t0 - ct0) * 8 : (t0 - ct0 + nt) * 8],
                        num_idxs=nt * P,
                        num_idxs_reg=nt * P,
                        elem_size=F,
                        queue_num=call_counter[0] % NUM_QUEUES,
                        single_packet=True,
                    )
                    call_counter[0] += 1
                    for t in range(nt):
                        tiles_map[t0 + t] = (s_t, m_t, t)
                    # emit groups fully covered by gathered tiles
                    while gi < len(groups) and (
                        groups[gi][1] + groups[gi][2] <= t0 + nt
                    ):
                        emit_group(*groups[gi])
                        gi += 1
                while gi < len(groups):
                    emit_group(*groups[gi])
                    gi += 1
            for w in range(W):
                if first_k[w] < 0:  # window with no in-edges at all
                    nc.vector.memset(agg[:, w, :], 0.0)
                    consume(w)

        # ---- phase A: h0 = feat * norm -> AG, pipelined per quarter ----
        # feat (fp32) staged per quarter through fpool, DVE-cast into tx0.
        for q in range(NK):
            q0, q1 = int(cfg.qoff[q]), int(cfg.qoff[q + 1])
            ncols = (q1 - q0) * F
            stage = fpool.tile([P, max(cfg.Wq) * F], dt.float32, tag="feat")
            nc.sync.dma_start(
                out=stage[:, :ncols], in_=feat_pw.ap()[:, q0 * F : q1 * F]
            )
            nc.vector.tensor_copy(
                out=tx0_flat[:, q0 * F : q1 * F], in_=stage[:, :ncols]
            )
            build_h_quarter(tx0, 0, q)

        # ---- phase B: Tx1 = spmm(h0) * nl - Tx0 ----
        qleft1 = list(cfg.Wq)  # windows still unconsumed per quarter

        def consume1(w):
            nc.vector.scalar_tensor_tensor(
                out=tx1[:, w, :],
                in0=agg[:, w, :],
                scalar=nl[:, w : w + 1],
                in1=tx0[:, w, :],
                op0=mybir.AluOpType.mult,
                op1=mybir.AluOpType.subtract,
            )
            q = cfg.quarter_of_window(w)
            qleft1[q] -= 1
            if qleft1[q] == 0 and os.environ.get("CHEB_NOOVERLAP", "0") != "1":
                # whole quarter of tx1 finalized: ship its h1 piece now so
                # spmm2's chunk-q gathers can start while spmm1 still runs.
                build_h_quarter(tx1, 1, q)

        spmm(h_full[0], consume1)
        if os.environ.get("CHEB_NOOVERLAP", "0") == "1":
            for q in range(NK):
                build_h_quarter(tx1, 1, q)

        # ---- phase C: Tx2 = 2*(spmm(h1)*nl - Tx1) - Tx0 ;
        #               out = Tx0@W0 + Tx1@W1 + Tx2@W2 + b ----
        def consume2(w):
            tmp = vpool.tile([P, F], dt.float32, tag="tmp")
            nc.vector.scalar_tensor_tensor(
                out=tmp[:],
                in0=agg[:, w, :],
                scalar=nl[:, w : w + 1],
                in1=tx1[:, w, :],
                op0=mybir.AluOpType.mult,
                op1=mybir.AluOpType.subtract,
            )
            tx2 = vpool.tile([P, F], dt.bfloat16, tag="tx2")
            nc.vector.scalar_tensor_tensor(
                out=tx2[:],
                in0=tmp[:],
                scalar=2.0,
                in1=tx0[:, w, :],
                op0=mybir.AluOpType.mult,
                op1=mybir.AluOpType.subtract,
            )
            rst = rpool.tile([P, F], dt.float32, space="PSUM", tag="rst")
            for k, txk in enumerate(
                [tx0[:, w, :], tx1[:, w, :], tx2[:]]
            ):
                tp = ppool.tile([P, F], dt.bfloat16, space="PSUM", tag="tp")
                nc.tensor.transpose(tp[:], txk, ident_b[:])
                tkT = vpool.tile([P, F], dt.bfloat16, tag="tkT")
                nc.vector.tensor_copy(out=tkT[:], in_=tp[:])
                nc.tensor.matmul(
                    rst[:], lhsT=tkT[:], rhs=wsb[k][:],
                    start=(k == 0), stop=(k == 2),
                )
            ob = opool.tile([P, F], dt.float32, tag="ob")
            nc.vector.tensor_tensor(
                out=ob[:], in0=rst[:], in1=b128[:], op=mybir.AluOpType.add
            )
            nc.sync.dma_start(out=out_pw.ap()[:, w * F : (w + 1) * F], in_=ob[:])

        spmm(h_full[1], consume2)

    nc.compile()
    return nc


def make_in_maps(cfg, deg, pre, feat, W_arr, b, lam):
    NC, NPC, W = cfg.NC, cfg.NPC, cfg.W
    feat = np.asarray(feat, dtype=np.float32)
    wmat = np.asarray(W_arr, dtype=np.float32).reshape(K * F, F)
    bvec = np.asarray(b, dtype=np.float32).reshape(1, F)
    lam2 = np.asarray(lam, dtype=np.float32).reshape(1, 1)
    in_maps = []
    for c in range(NC):
        base = c * NPC
        in_maps.append(
            {
                "feat_pw": pack_pw(feat[base : base + NPC], W),
                "deg_pw": pack_pw(
                    deg[base : base + NPC, None], W, fill=1.0
                ).reshape(P, W),
                "lam_d": lam2,
                "wmat_d": wmat,
                "bvec_d": bvec,
                "ident_d": np.eye(P, dtype=np.float32),
                "idx_d": pre[c]["idx16"],
                "s_d": pre[c]["s"],
            }
        )
    return in_maps


_CACHE = {}


def _get_program(cfg, B):
    key = (cfg.N, cfg.E, cfg.NC, cfg.GMAX, B.tobytes())
    if key not in _CACHE:
        _CACHE[key] = build_program(cfg, B)
    return _CACHE[key]


def kernel(feat, src, dst, W, b, lambda_max):
    cfg = FULL
    B, deg, pre = preprocess(cfg, src, dst)
    nc = _get_program(cfg, B)
    in_maps = make_in_maps(cfg, deg, pre, feat, W, b, lambda_max)
    res = run_bass_kernel_spmd(
        nc,
        in_maps,
        core_ids=list(range(cfg.NC)),
        trace=os.environ.get("CHEB_TRACE", "0") == "1",
    )
    outs = []
    for c in range(cfg.NC):
        outs.append(unpack_pw(res.results[c]["out_pw"], cfg.W, cfg.NPC, F))
    out = np.concatenate(outs, axis=0).astype(np.float32)
    kernel.last_exec_time_ns = res.exec_time_ns
    return out



# revision 22
# speedup vs baseline: 1.1525x; 1.1525x over previous
"""ChebConv (K=3) GNN message-passing kernel for Trainium2, 8 NeuronCores.

Strategy (graph/data parallel, dst-partitioned):
  - Nodes split into 8 contiguous ranges (12500/core); each core owns the
    output rows and in-edges of its range. Node storage is window-packed:
    node r of a core sits at SBUF [p=r%128, w=r//128]; the shared h table in
    HBM uses packed row index c*12544 + p*W + w so a full-slice DMA is one
    contiguous copy.
  - Edges are grouped per core by (src 32768-row chunk, 128-node dst window)
    with uniform cross-core tile budgets so the SPMD program is identical on
    all cores. Tiles are chunk-major, enabling big gather calls.
  - Message gather: 8192-index SWDGE dma_gather calls (64 tiles per call,
    int16 indices) pull h[src] rows (bf16, 256B) from the HBM table into
    [128 edges x F] SBUF tiles. Big calls amortize the ~1us/call Q7
    descriptor-generation overhead that dominated the previous version.
  - Aggregation: per 128-edge tile, a host-built one-hot S tile (fp8)
    maps edges to window rows via the TensorEngine; per-window PSUM partials
    are flushed into an fp16 per-core accumulator across chunks.
  - Chebyshev recurrence in bf16 on-chip; h tables rebuilt per hop and
    exchanged with AllGather.
Host-side work is limited to graph restructuring (edge partition / sort /
padding, degree counting, one-hot layout) - all floating-point math on node
features happens on device.
"""

import os
import sys

import numpy as np

sys.path.insert(0, "/opt/trn_rl_repo")

from contextlib import ExitStack


def _ensure_ntff_hook():
    """The agent image's antenv lacks axon_hooks; synthesize the module so
    run_bass_kernel_spmd(trace=True) can reach the NTFF profiler."""
    import types

    if "antenv.axon_hooks" in sys.modules:
        return
    try:
        import trn_agent_boot.trn_boot as tb

        hook = tb._ntff_profile_via_ctypes("/opt/axon/libaxon_pjrt.so")
    except Exception:
        hook = None
    mod = types.ModuleType("antenv.axon_hooks")
    state = {"hook": hook}
    mod.get_axon_ntff_profile_hook = lambda: state["hook"]
    mod.set_axon_ntff_profile_hook = lambda h: state.update(hook=h)
    sys.modules["antenv.axon_hooks"] = mod


_ensure_ntff_hook()

import concourse.bacc as bacc
import concourse.bass as bass
import concourse.tile as tile
from concourse import mybir
from concourse.bass_utils import run_bass_kernel_spmd

P = 128
F = 128
K = 3


class Cfg:
    def __init__(self, N, E, n_cores, ch=32768, gmax=None,
                 s_dtype=mybir.dt.float8e4):
        self.N = N
        self.E = E
        self.NC = n_cores
        assert N % n_cores == 0
        self.NPC = N // n_cores
        self.W = (self.NPC + P - 1) // P      # windows per core
        self.WP = self.W * P                  # packed rows per core
        self.NPACK = self.NC * self.WP        # packed rows total
        self.CH = ch                          # chunk rows (int16 idx range)
        self.NK = (self.NPACK + ch - 1) // ch
        self.chunkbase = np.arange(self.NK + 1) * ch
        self.chunkbase[-1] = self.NPACK
        self.GMAX = gmax if gmax is not None else int(
            os.environ.get("CHEB_GMAX", "8")
        )
        self.s_dtype = s_dtype


FULL = Cfg(100000, 3200000, 8)
NUM_QUEUES = int(os.environ.get("CHEB_QUEUES", "4"))


def packed_index(cfg, u):
    """Global node id -> packed h-table row index."""
    c = u // cfg.NPC
    r = u - c * cfg.NPC
    return c * cfg.WP + (r % P) * cfg.W + (r // P)


def preprocess(cfg, src, dst):
    """Partition edges by (dst core, src chunk, dst window); assign 128-edge
    tile slots with static per-(chunk, window) budgets (max over cores).

    Returns (B, deg, per_core):
      B:   [NK, W] int tile budgets (chunk-major tile layout, w-minor)
      deg: [N] float32 in-degrees
      per_core[c]: idx16 [128, TOT*8] int16 stream, s [128, TOT*F] one-hot
    """
    N, NC, NPC, W, NK = cfg.N, cfg.NC, cfg.NPC, cfg.W, cfg.NK
    src = np.asarray(src).astype(np.int64)
    dst = np.asarray(dst).astype(np.int64)
    deg = np.bincount(dst, minlength=N).astype(np.float32)

    gs_all = packed_index(cfg, src)

    cores = []
    cnts = np.zeros((NC, NK, W), dtype=np.int64)
    for c in range(NC):
        base = c * NPC
        sel = (dst >= base) & (dst < base + NPC)
        eg = gs_all[sel]
        rd = dst[sel] - base
        wof = rd >> 7
        dl = rd & 127
        ch = np.searchsorted(cfg.chunkbase, eg, side="right") - 1
        loc = eg - cfg.chunkbase[ch]
        order = np.lexsort((wof, ch))
        eg, wof, dl, ch, loc = (
            eg[order], wof[order], dl[order], ch[order], loc[order]
        )
        np.add.at(cnts[c], (ch, wof), 1)
        cores.append((loc, wof, dl, ch))

    B = np.ceil(cnts.max(axis=0) / P).astype(np.int64)  # [NK, W]
    flat = B.reshape(-1)
    toff = np.concatenate([[0], np.cumsum(flat)[:-1]]).reshape(NK, W)
    TOT = int(flat.sum())

    s_np_dtype = mybir.dt.np(cfg.s_dtype)
    one = np.ones(1, dtype=s_np_dtype)[0]
    per_core = []
    for c in range(NC):
        loc, wof, dl, ch = cores[c]
        seg_sizes = cnts[c].reshape(-1)
        seg_starts = np.concatenate([[0], np.cumsum(seg_sizes)[:-1]])
        seg_id = ch * W + wof
        rank = np.arange(len(loc)) - seg_starts[seg_id]
        gslot = toff[ch, wof] * P + rank

        idx16 = np.zeros((16, TOT * 8), dtype=np.int16)
        idx16[gslot % 16, gslot // 16] = loc.astype(np.int16)
        idx16 = np.tile(idx16, (8, 1))

        s_stream = np.zeros((P, TOT * F), dtype=s_np_dtype)
        s_stream[gslot % P, (gslot // P) * F + dl] = one
        per_core.append({"idx16": idx16, "s": s_stream})
    return B, deg, per_core


def pack_pw(arr_slice, W, fill=0.0):
    """[NPC, ...] node-major -> [128, W * inner] partition/window packed."""
    NPC = arr_slice.shape[0]
    inner = arr_slice.shape[1] if arr_slice.ndim > 1 else 1
    pad_rows = W * P - NPC
    a = arr_slice.reshape(NPC, inner)
    if pad_rows:
        a = np.concatenate(
            [a, np.full((pad_rows, inner), fill, dtype=a.dtype)], axis=0
        )
    return np.ascontiguousarray(
        a.reshape(W, P, inner).transpose(1, 0, 2).reshape(P, W * inner)
    )


def unpack_pw(a, W, NPC, inner):
    """[128, W * inner] -> [NPC, inner]."""
    return np.ascontiguousarray(
        a.reshape(P, W, inner).transpose(1, 0, 2).reshape(W * P, inner)[:NPC]
    )


def plan_calls(cfg, B):
    """Static gather-call plan. Returns per chunk k:
    (ct0, ct1, calls, groups) with calls = fixed-size [(t0, nt)] runs and
    groups = [(w, ts, u)]; a group may span up to 3 consecutive calls."""
    NK, W = B.shape
    running = 0
    plans = []
    for k in range(NK):
        ct0 = running
        groups = []
        for w in range(W):
            if B[k, w] > 0:
                groups.append((w, int(running), int(B[k, w])))
                running += int(B[k, w])
        ct1 = running
        calls = [
            (t0, min(cfg.GMAX, ct1 - t0))
            for t0 in range(ct0, ct1, cfg.GMAX)
        ]
        plans.append((ct0, ct1, calls, groups))
    return plans


def build_program(cfg, B):
    NC, W, NK = cfg.NC, cfg.W, cfg.NK
    B = np.asarray(B)
    plans = plan_calls(cfg, B)
    TOT = int(B.sum())
    CT_MAX = max(ct1 - ct0 for ct0, ct1, _, _ in plans)
    GMAX = cfg.GMAX
    dt = mybir.dt
    nc = bacc.Bacc(
        "TRN2",
        target_bir_lowering=False,
        debug=False,
        enable_asserts=False,
        num_devices=NC,
        num_swdge_queues=NUM_QUEUES,
        # 32KB/partition desc-ring carveout -> 256 descs per SWDGE queue ring
        # (default 16KB -> 128). A 1024-idx gather call writes 65 descs per
        # DMA ring; 128-deep rings fit only one call per queue, so the Q7
        # desc-gen stalls in await_space whenever the DMA engines lag a call.
        dynamic_dma_scratch_size=int(os.environ.get("CHEB_DSCRATCH", "32768")),
    )

    feat_pw = nc.dram_tensor("feat_pw", [P, W * F], dt.float32, kind="ExternalInput")
    deg_pw = nc.dram_tensor("deg_pw", [P, W], dt.float32, kind="ExternalInput")
    lam_d = nc.dram_tensor("lam_d", [1, 1], dt.float32, kind="ExternalInput")
    wmat_d = nc.dram_tensor("wmat_d", [K * F, F], dt.float32, kind="ExternalInput")
    bvec_d = nc.dram_tensor("bvec_d", [1, F], dt.float32, kind="ExternalInput")
    ident_d = nc.dram_tensor("ident_d", [P, P], dt.float32, kind="ExternalInput")
    idx_d = nc.dram_tensor("idx_d", [P, TOT * 8], dt.int16, kind="ExternalInput")
    s_d = nc.dram_tensor("s_d", [P, TOT * F], cfg.s_dtype, kind="ExternalInput")
    out_pw = nc.dram_tensor("out_pw", [P, W * F], dt.float32, kind="ExternalOutput")

    h_slice = [
        nc.dram_tensor(f"h{i}_slice", [cfg.WP, F], dt.bfloat16, kind="Internal")
        for i in range(2)
    ]
    h_full = [
        nc.dram_tensor(
            f"h{i}_full", [cfg.NPACK, F], dt.bfloat16, kind="Internal",
            addr_space="Shared",
        )
        for i in range(2)
    ]
    groups_cc = [list(range(NC))]

    with tile.TileContext(nc) as tc, ExitStack() as ctx:
        const = ctx.enter_context(tc.tile_pool(name="const", bufs=1))
        ipool = ctx.enter_context(tc.tile_pool(name="ipool", bufs=2))
        # Tile rotates Pool-DMA completion sems over 8 DMASW lanes (emission
        # order). bufs=6 keeps at most 6 gathers in flight -> no two live
        # gathers share a lane. bufs=8 DID race (two in-flight gathers alias
        # a lane; matmuls read against the wrong completion: rel err 0.017).
        gbufs = int(os.environ.get("CHEB_GBUFS", "6"))
        spool = ctx.enter_context(tc.tile_pool(name="spool", bufs=gbufs))
        mpool = ctx.enter_context(tc.tile_pool(name="mpool", bufs=gbufs))
        vpool = ctx.enter_context(tc.tile_pool(name="vpool", bufs=3))
        opool = ctx.enter_context(tc.tile_pool(name="opool", bufs=3))
        apool = ctx.enter_context(tc.tile_pool(name="apool", bufs=2, space="PSUM"))
        ppool = ctx.enter_context(tc.tile_pool(name="ppool", bufs=2, space="PSUM"))
        rpool = ctx.enter_context(tc.tile_pool(name="rpool", bufs=2, space="PSUM"))
        mppool = ctx.enter_context(tc.tile_pool(name="mppool", bufs=1, space="PSUM"))

        # ---- constants / persistent state ----
        # NOTE: keep nc.gpsimd DMA usage to dma_gather ONLY — Tile rotates
        # SWDGE sem lanes per Pool-DMA in emission order and each lane is
        # locked to one queue; any extra Pool DMA breaks the queue rotation.
        tx0 = const.tile([P, W, F], dt.bfloat16)
        tx1 = const.tile([P, W, F], dt.bfloat16)
        scratch = const.tile([P, W, F], dt.float16)  # agg (fp16) / hb (bf16)
        agg = scratch[:]
        hb = scratch[:].bitcast(dt.bfloat16)
        tx0_flat = tx0[:].rearrange("p w f -> p (w f)")
        wsb = []
        for k in range(K):
            wtmp = vpool.tile([P, F], dt.float32, tag="wtmp")
            nc.sync.dma_start(
                out=wtmp[:], in_=wmat_d.ap()[k * F : (k + 1) * F, :]
            )
            wk = const.tile([P, F], dt.bfloat16, tag=f"wsb{k}")
            nc.vector.tensor_copy(out=wk[:], in_=wtmp[:])
            wsb.append(wk)
        ident = const.tile([P, P], dt.float32)
        nc.sync.dma_start(out=ident[:], in_=ident_d.ap())
        ident_b = const.tile([P, P], dt.bfloat16)
        nc.vector.tensor_copy(out=ident_b[:], in_=ident[:])
        ones_row = const.tile([1, P], dt.float32)
        nc.vector.memset(ones_row[:], 1.0)
        bvec_sb = const.tile([1, F], dt.float32)
        nc.sync.dma_start(out=bvec_sb[:], in_=bvec_d.ap())

        deg_sb = const.tile([P, W], dt.float32)
        nc.sync.dma_start(out=deg_sb[:], in_=deg_pw.ap())
        norm = const.tile([P, W], dt.float32)
        b128 = const.tile([P, F], dt.float32)
        nl = const.tile([P, W], dt.float32)  # norm * 2/lambda

        rec_deg = const.tile([P, W], dt.float32)
        nc.vector.reciprocal(rec_deg[:], deg_sb[:])
        nc.scalar.activation(
            norm[:], rec_deg[:], mybir.ActivationFunctionType.Sqrt
        )

        lam_sb = const.tile([1, 1], dt.float32)
        nc.sync.dma_start(out=lam_sb[:], in_=lam_d.ap())
        lam_half = const.tile([1, 1], dt.float32)
        nc.vector.tensor_scalar(
            lam_half[:], lam_sb[:], 0.5, None, mybir.AluOpType.mult
        )
        lap_sc = const.tile([1, 1], dt.float32)  # 2 / lambda_max
        nc.vector.reciprocal(lap_sc[:], lam_half[:])

        # broadcast 2/lambda to all partitions: ones[1,P]^T @ lap[1,1]
        lap_ps = mppool.tile([P, 1], dt.float32, space="PSUM", tag="mpsum")
        nc.tensor.matmul(
            lap_ps[:], lhsT=ones_row[:], rhs=lap_sc[:], start=True, stop=True
        )
        lap_bc = const.tile([P, 1], dt.float32)
        nc.vector.tensor_copy(out=lap_bc[:], in_=lap_ps[:])
        # bias broadcast to all partitions
        b_ps = mppool.tile([P, F], dt.float32, space="PSUM", tag="mpsum")
        nc.tensor.matmul(
            b_ps[:], lhsT=ones_row[:], rhs=bvec_sb[:], start=True, stop=True
        )
        nc.vector.tensor_copy(out=b128[:], in_=b_ps[:])

        nc.vector.tensor_scalar(
            nl[:], norm[:], lap_bc[:], None, mybir.AluOpType.mult
        )

        norm_b = norm[:].unsqueeze(-1).broadcast_to([P, W, F])

        def build_h(tx_tile, i):
            # h = tx * norm, one broadcast DVE op + one contiguous DMA
            nc.vector.tensor_tensor(
                out=hb, in0=tx_tile[:], in1=norm_b, op=mybir.AluOpType.mult
            )
            nc.sync.dma_start(
                out=h_slice[i].ap(), in_=hb.rearrange("p w f -> p (w f)")
            )

        def allgather(i):
            if os.environ.get("CHEB_NOCC", "0") == "1":
                for blk in range(0, cfg.WP, P):
                    tmp = vpool.tile([P, F], dt.bfloat16, tag="agtmp")
                    nc.sync.dma_start(
                        out=tmp[:], in_=h_slice[i].ap()[blk : blk + P, :]
                    )
                    nc.sync.dma_start(
                        out=h_full[i].ap()[blk : blk + P, :], in_=tmp[:]
                    )
                return
            nc.gpsimd.collective_compute(
                "AllGather",
                mybir.AluOpType.bypass,
                replica_groups=groups_cc,
                ins=[h_slice[i].ap()],
                outs=[h_full[i].ap()],
            )

        call_counter = [0]

        # first/last chunk containing tiles for each window (-1 = no edges)
        first_k = [-1] * W
        last_k = [-1] * W
        for w in range(W):
            ks = [k for k in range(NK) if B[k, w] > 0]
            if ks:
                first_k[w], last_k[w] = ks[0], ks[-1]

        def spmm(h_full_t, consume):
            """agg (fp16 scratch) = sum over edges; consume(w) after the
            window's last-chunk flush."""
            for k in range(NK):
                ct0, ct1, calls, groups = plans[k]
                nt_chunk = ct1 - ct0
                if nt_chunk == 0:
                    continue
                lo = int(cfg.chunkbase[k])
                hi = int(cfg.chunkbase[k + 1])
                i_t = ipool.tile([P, CT_MAX * 8], dt.int16, tag="i16")
                nc.sync.dma_start(
                    out=i_t[:, : nt_chunk * 8],
                    in_=idx_d.ap()[:, ct0 * 8 : ct1 * 8],
                )
                tiles_map = {}  # global tile idx -> (s_tile, m_tile, off)
                gi = 0  # next group to emit

                def emit_group(w, ts, u):
                    ps = apool.tile([P, F], dt.float32, space="PSUM", tag="agg")
                    for t in range(u):
                        s_t, m_t, off = tiles_map[ts + t]
                        nc.tensor.matmul(
                            ps[:],
                            lhsT=s_t[:, off, :],
                            rhs=m_t[:, off, :],
                            start=(t == 0),
                            stop=(t == u - 1),
                        )
                    if k == first_k[w]:
                        nc.vector.tensor_copy(out=agg[:, w, :], in_=ps[:])
                    else:
                        nc.vector.tensor_tensor(
                            out=agg[:, w, :], in0=agg[:, w, :], in1=ps[:],
                            op=mybir.AluOpType.add,
                        )
                    if k == last_k[w]:
                        consume(w)

                for (t0, nt) in calls:
                    s_t = spool.tile([P, GMAX, F], cfg.s_dtype, tag="s")
                    nc.sync.dma_start(
                        out=s_t[:, :nt, :].rearrange("p t f -> p (t f)"),
                        in_=s_d.ap()[:, t0 * F : (t0 + nt) * F],
                    )
                    m_t = mpool.tile([P, GMAX, F], dt.bfloat16, tag="msg")
                    nc.gpsimd.dma_gather(
                        out_ap=m_t[:, :nt, :],
                        in_ap=h_full_t.ap()[lo:hi, :],
                        idxs_ap=i_t[:, (t0 - ct0) * 8 : (t0 - ct0 + nt) * 8],
                        num_idxs=nt * P,
                        num_idxs_reg=nt * P,
                        elem_size=F,
                        queue_num=call_counter[0] % NUM_QUEUES,
                        single_packet=True,
                    )
                    call_counter[0] += 1
                    for t in range(nt):
                        tiles_map[t0 + t] = (s_t, m_t, t)
                    # emit groups fully covered by gathered tiles
                    while gi < len(groups) and (
                        groups[gi][1] + groups[gi][2] <= t0 + nt
                    ):
                        emit_group(*groups[gi])
                        gi += 1
                while gi < len(groups):
                    emit_group(*groups[gi])
                    gi += 1
            for w in range(W):
                if first_k[w] < 0:  # window with no in-edges at all
                    nc.vector.memset(agg[:, w, :], 0.0)
                    consume(w)

        # ---- phase A: h0 = feat * norm -> allgather ----
        # feat (fp32) -> tx0 (bf16): stage halves through scratch, DVE-cast
        stage32 = scratch[:].rearrange("p w f -> p (w f)").bitcast(dt.float32)
        half = (W * F) // 2
        for h0 in (0, half):
            hi2 = min(h0 + half, W * F)
            nc.sync.dma_start(
                out=stage32[:, : hi2 - h0], in_=feat_pw.ap()[:, h0:hi2]
            )
            nc.vector.tensor_copy(
                out=tx0_flat[:, h0:hi2], in_=stage32[:, : hi2 - h0]
            )
        build_h(tx0, 0)
        allgather(0)

        # ---- phase B: Tx1 = spmm(h0) * nl - Tx0 ----
        def consume1(w):
            nc.vector.scalar_tensor_tensor(
                out=tx1[:, w, :],
                in0=agg[:, w, :],
                scalar=nl[:, w : w + 1],
                in1=tx0[:, w, :],
                op0=mybir.AluOpType.mult,
                op1=mybir.AluOpType.subtract,
            )

        # ---- hoisted W0 term: base_w = Tx0 @ W0 + b, computed while the
        # spmm1 gathers keep GpSimd busy and PE is otherwise idle ----
        base_t = const.tile([P, W, F], dt.float16)

        def emit_base(w):
            tp = ppool.tile([P, F], dt.bfloat16, space="PSUM", tag="tp")
            nc.tensor.transpose(tp[:], tx0[:, w, :], ident_b[:])
            tkT = vpool.tile([P, F], dt.bfloat16, tag="tkT")
            nc.vector.tensor_copy(out=tkT[:], in_=tp[:])
            bps = rpool.tile([P, F], dt.float32, space="PSUM", tag="rst")
            nc.tensor.matmul(
                bps[:], lhsT=tkT[:], rhs=wsb[0][:], start=True, stop=True
            )
            nc.vector.tensor_tensor(
                out=base_t[:, w, :], in0=bps[:], in1=b128[:],
                op=mybir.AluOpType.add,
            )

        for w in range(W):
            emit_base(w)

        spmm(h_full[0], consume1)
        build_h(tx1, 1)
        allgather(1)

        # hoisted Tx1 transposes: run during spmm2's early chunks
        tx1T = const.tile([P, W, F], dt.bfloat16)
        for w in range(W):
            tp = ppool.tile([P, F], dt.bfloat16, space="PSUM", tag="tp")
            nc.tensor.transpose(tp[:], tx1[:, w, :], ident_b[:])
            nc.vector.tensor_copy(out=tx1T[:, w, :], in_=tp[:])

        # ---- phase C: Tx2 = 2*(spmm(h1)*nl - Tx1) - Tx0 ;
        #               out = base + Tx1@W1 + Tx2@W2 ----
        def consume2(w):
            tmp = vpool.tile([P, F], dt.float32, tag="tmp")
            nc.vector.scalar_tensor_tensor(
                out=tmp[:],
                in0=agg[:, w, :],
                scalar=nl[:, w : w + 1],
                in1=tx1[:, w, :],
                op0=mybir.AluOpType.mult,
                op1=mybir.AluOpType.subtract,
            )
            tx2 = vpool.tile([P, F], dt.bfloat16, tag="tx2")
            nc.vector.scalar_tensor_tensor(
                out=tx2[:],
                in0=tmp[:],
                scalar=2.0,
                in1=tx0[:, w, :],
                op0=mybir.AluOpType.mult,
                op1=mybir.AluOpType.subtract,
            )
            rst = rpool.tile([P, F], dt.float32, space="PSUM", tag="rst")
            nc.tensor.matmul(
                rst[:], lhsT=tx1T[:, w, :], rhs=wsb[1][:],
                start=True, stop=False,
            )
            tp = ppool.tile([P, F], dt.bfloat16, space="PSUM", tag="tp")
            nc.tensor.transpose(tp[:], tx2[:], ident_b[:])
            tkT = vpool.tile([P, F], dt.bfloat16, tag="tkT")
            nc.vector.tensor_copy(out=tkT[:], in_=tp[:])
            nc.tensor.matmul(
                rst[:], lhsT=tkT[:], rhs=wsb[2][:], start=False, stop=True
            )
            ob = opool.tile([P, F], dt.float32, tag="ob")
            nc.vector.tensor_tensor(
                out=ob[:], in0=rst[:], in1=base_t[:, w, :],
                op=mybir.AluOpType.add,
            )
            nc.sync.dma_start(out=out_pw.ap()[:, w * F : (w + 1) * F], in_=ob[:])

        spmm(h_full[1], consume2)

    nc.compile()
    return nc


def make_in_maps(cfg, deg, pre, feat, W_arr, b, lam):
    NC, NPC, W = cfg.NC, cfg.NPC, cfg.W
    feat = np.asarray(feat, dtype=np.float32)
    wmat = np.asarray(W_arr, dtype=np.float32).reshape(K * F, F)
    bvec = np.asarray(b, dtype=np.float32).reshape(1, F)
    lam2 = np.asarray(lam, dtype=np.float32).reshape(1, 1)
    in_maps = []
    for c in range(NC):
        base = c * NPC
        in_maps.append(
            {
                "feat_pw": pack_pw(feat[base : base + NPC], W),
                "deg_pw": pack_pw(
                    deg[base : base + NPC, None], W, fill=1.0
                ).reshape(P, W),
                "lam_d": lam2,
                "wmat_d": wmat,
                "bvec_d": bvec,
                "ident_d": np.eye(P, dtype=np.float32),
                "idx_d": pre[c]["idx16"],
                "s_d": pre[c]["s"],
            }
        )
    return in_maps


_CACHE = {}


def _get_program(cfg, B):
    key = (cfg.N, cfg.E, cfg.NC, cfg.GMAX, B.tobytes())
    if key not in _CACHE:
        _CACHE[key] = build_program(cfg, B)
    return _CACHE[key]


def kernel(feat, src, dst, W, b, lambda_max):
    cfg = FULL
    B, deg, pre = preprocess(cfg, src, dst)
    nc = _get_program(cfg, B)
    in_maps = make_in_maps(cfg, deg, pre, feat, W, b, lambda_max)
    res = run_bass_kernel_spmd(
        nc,
        in_maps,
        core_ids=list(range(cfg.NC)),
        trace=os.environ.get("CHEB_TRACE", "0") == "1",
    )
    outs = []
    for c in range(cfg.NC):
        outs.append(unpack_pw(res.results[c]["out_pw"], cfg.W, cfg.NPC, F))
    out = np.concatenate(outs, axis=0).astype(np.float32)
    kernel.last_exec_time_ns = res.exec_time_ns
    return out

